# revision 9
# baseline (speedup 1.0000x reference)
"""Trainium2 Bass kernel for nn_Attention_82540681494971.

Spatial self-attention block (LDM AttnBlock style, unscaled):
  qkv = conv1x1(x);  s = q^T k  [n x n] per (b,head);  attn = softmax(s, axis=-1)
  out[d,m] = sum_n v[d,n] attn[n,m];  y = conv1x1(out)

Shapes: B=4, C=64, H=W=64 -> n=4096 tokens, HEAD=4, d=16.

Sharding: 8 cores, core c handles batch b=c//2 and heads (0,1) if c%2==0
else (2,3). Each core computes a partial projection output over its two
heads' channels; host sums the two partials per batch and adds proj bias.

Key algebra: attn[n,m] = E[n,m]/rowsum[n] with E=exp(s). Since the AV
contraction runs over n (the softmax row index), fold 1/rowsum into v:
  out[d,m] = sum_n (v[d,n]*rinv[n]) E[n,m]
so the big E matrix never needs normalizing. Scores are exact-fp32-grade
via a 3-term bf16 split (q=q_hi+q_lo, k=k_hi+k_lo, drop lo*lo):
  s = [q_hi;q_lo;q_hi]^T [k_hi;k_hi;k_lo]   (K=48 stacked, 1 cyc/row)

v2 design (ACT-roofline targeted):
  - ACT does exp ONLY: rowsums split across DVE and GpSimd reduces over
    the bf16 E tile (no ACT accum_out reads).
  - AV runs 4 concurrent PSUM-accumulation chains in PE col-groups
    (tile_position=(0,32s)); strip s owns m-chunks {s, 4+s}. The two
    [112,512] PSUM accumulators persist across all groups of a head.
  - qkv matmuls in bf16 via a host-side 3-term weight split (W_hi x_hi +
    W_hi x_lo + W_lo x_hi, bias exact via hi/lo bias rows) - denser PE
    stream than fp32, warms the HAM clock gate early.
  - v matmuls and head-0 projection woven into phase 1, using the idle
    3rd PSUM bank of chunk-3 score slots as scratch.
  - Projection col-tiled over 4 row-groups in bf16, split per head:
    h0 during h1's steady state, h1 at the tail (y = y0 + yp1 on DVE).
  - PSUM: 6 banks scores (2 x [128,1536] double-buffer) + 2 banks AV.
"""

import numpy as np
from contextlib import ExitStack

import concourse.bass as bass
import concourse.mybir as mybir
import concourse.tile as tile
from concourse import bacc
from concourse.bass import ts, ds
from concourse.bass_utils import run_bass_kernel_spmd

F32 = mybir.dt.float32
BF16 = mybir.dt.bfloat16
AF = mybir.ActivationFunctionType

B, C, HEAD, D = 4, 64, 4, 16
N = 4096          # tokens = H*W
NT = 128          # n-tile (partition) size
NTILES = N // NT  # 32
MC = 512          # matmul free-dim chunk
MCN = N // MC     # 8 m-chunks
SCH = (1536, 1536, 1024)  # scores/exp PSUM chunking (3 banks/slot, 2 bufs)
GROUPS = [4] * 7 + [2, 2]  # AV group sizes (short tail for a fast finish)


def _body(tc, y, x1, wqh, wql, wkh, wkl, wv, wp0, wp1):
    nc = tc.nc
    ctx = ExitStack()
    with ctx:
        pp = ctx.enter_context(tc.tile_pool(name="persist", bufs=1))
        cp = ctx.enter_context(tc.tile_pool(name="consts", bufs=1))

        # ---- warm the exp table while DMAs run ----
        zz = pp.tile([NT, 1], F32)
        zz2 = pp.tile([NT, 1], F32)
        nc.gpsimd.memset(zz[:], 0.0)
        nc.scalar.activation(zz2[:], zz[:], AF.Exp)

        # ---- constants ----
        wf = {}
        for name, src in (("qh", wqh), ("ql", wql), ("kh", wkh), ("kl", wkl)):
            t = cp.tile([C + 1, 2 * D], F32, tag="wf", name=f"wf_{name}",
                        bufs=4)
            nc.sync.dma_start(t[:], src[:])
            wf[name] = t
        wv_t = cp.tile([C + 1, 2 * D], F32)
        nc.sync.dma_start(wv_t[:], wv[:])
        wp0_f = cp.tile([112, C], F32)
        wp1_f = cp.tile([112, C], F32)
        nc.sync.dma_start(wp0_f[:], wp0[:])
        nc.sync.dma_start(wp1_f[:], wp1[:])
        wb = {}
        for name in ("qh", "ql", "kh", "kl"):
            t = cp.tile([C + 1, 2 * D], BF16, tag="wb", name=f"wb_{name}",
                        bufs=4)
            nc.vector.tensor_copy(t[:], wf[name][:])
            wb[name] = t
        wp_t = [cp.tile([112, C], BF16, tag="wpb", name=f"wpb{h}", bufs=2)
                for h in range(2)]
        nc.vector.tensor_copy(wp_t[0][:], wp0_f[:])
        nc.vector.tensor_copy(wp_t[1][:], wp1_f[:])

        # ---- persistent SBUF ----
        x1_t = pp.tile([C + 1, N], F32)   # kept for v matmuls in phase 1
        qsp = pp.tile([112, 2 * N], BF16)
        ksp = pp.tile([112, 2 * N], BF16)
        vT_sb = pp.tile([NT, NTILES * 2 * D], F32)  # per n-tile [128,32] h0|h1
        out_sb = [pp.tile([112, 2 * MC], BF16, tag=f"osb{h}", name=f"osb{h}")
                  for h in range(2)]
        y_sb = pp.tile([C, N], F32)    # head-0 projection partial
        y2_sb = pp.tile([C, N], F32)   # final y = y_sb + yp1


        # ---- phase 0: q,k in bf16 (3-term) + hi/lo split + K=48 stacks ----
        with (
            tc.tile_pool(name="x1p", bufs=1) as xp,
            tc.tile_pool(name="qkf", bufs=1) as qf,
            tc.tile_pool(name="spl", bufs=1) as spl,
            tc.tile_pool(name="p0psum", bufs=5, space="PSUM") as p0,
        ):
            for i in range(8):  # parallel DMA queues
                nc.sync.dma_start(x1_t[:, ts(i, N // 8)], x1[:, ts(i, N // 8)])
            x_hi = xp.tile([C + 1, N], BF16)
            x_lo = xp.tile([C + 1, N], BF16)
            for i in range(4):
                sl = ts(i, N // 4)
                nc.vector.tensor_copy(x_hi[:, sl], x1_t[:, sl])
                eng = nc.vector if i < 2 else nc.gpsimd
                eng.tensor_sub(x_lo[:, sl], x1_t[:, sl], x_hi[:, sl])

            for which, dst, dup in (("q", qsp, 2), ("k", ksp, 1)):
                w_hi, w_lo = wb[which[0] + "h"], wb[which[0] + "l"]
                f32_sb = qf.tile([D, 2 * N], F32, tag="qkf32")
                for h in range(2):
                    for mc in range(MCN):
                        ps = p0.tile([D, MC], F32, tag="p0")
                        nc.tensor.matmul(
                            ps[:], w_hi[:, ts(h, D)], x_hi[:, ts(mc, MC)],
                            start=True, stop=False)
                        nc.tensor.matmul(
                            ps[:], w_hi[:, ts(h, D)], x_lo[:, ts(mc, MC)],
                            start=False, stop=False)
                        nc.tensor.matmul(
                            ps[:], w_lo[:, ts(h, D)], x_hi[:, ts(mc, MC)],
                            start=False, stop=True)
                        dst_ap = f32_sb[:, ds(h * N + mc * MC, MC)]
                        if mc % 2 == 0:
                            nc.vector.tensor_copy(dst_ap, ps[:])
                        else:
                            nc.scalar.copy(dst_ap, ps[:])
                hi_t = spl.tile([D, 2 * N], BF16, tag="hi")
                lo_t = spl.tile([D, 2 * N], BF16, tag="lo")
                # split engines: q -> ACT casts + DVE subs; k -> gpsimd
                if which == "q":
                    nc.scalar.copy(hi_t[:, 0:N], f32_sb[:, 0:N])
                    nc.scalar.copy(hi_t[:, N:2 * N], f32_sb[:, N:2 * N])
                    nc.vector.tensor_sub(lo_t[:, 0:N], f32_sb[:, 0:N],
                                         hi_t[:, 0:N])
                    nc.vector.tensor_sub(lo_t[:, N:2 * N], f32_sb[:, N:2 * N],
                                         hi_t[:, N:2 * N])
                else:
                    nc.gpsimd.tensor_copy(hi_t[:, 0:N], f32_sb[:, 0:N])
                    nc.gpsimd.tensor_copy(hi_t[:, N:2 * N], f32_sb[:, N:2 * N])
                    nc.gpsimd.tensor_sub(lo_t[:, 0:N], f32_sb[:, 0:N],
                                         hi_t[:, 0:N])
                    nc.gpsimd.tensor_sub(lo_t[:, N:2 * N], f32_sb[:, N:2 * N],
                                         hi_t[:, N:2 * N])
                # assemble K=48 stack via SBUF->SBUF DMA
                lo_block = 1 if which == "q" else 2
                for i in range(4):
                    sl = ts(i, N // 2)
                    for b0 in (0, 64):
                        nc.sync.dma_start(dst[ds(b0, D), sl], hi_t[:, sl])
                        nc.sync.dma_start(
                            dst[ds(b0 + lo_block * D, D), sl], lo_t[:, sl])
                        nc.sync.dma_start(
                            dst[ds(b0 + dup * D, D), sl], hi_t[:, sl])

        # ---- phase 1: attention, software-pipelined ----
        # Per n-tile: 3 score-chunk matmul bursts feed 3 exp ACTIVATEs (ACT
        # exp-only). Rowsum split DVE+GpSimd from the bf16 E tile. Queued
        # work (AV rounds of the previous group, accum evacs, head-0
        # projection) pops once per n-tile with the chunk-3 psum slot's
        # idle 3rd bank as scratch. v matmuls for n-tile 4i..4i+3 run in
        # n-tile i's chunk-3 slot (first 8 n-tiles of head 0).
        with (
            tc.tile_pool(name="ep", bufs=10) as ep,
            tc.tile_pool(name="rp", bufs=6) as rp,
            tc.tile_pool(name="vp", bufs=10) as vp,
            tc.tile_pool(name="sapsum", bufs=2, space="PSUM") as sp,
            tc.tile_pool(name="avpsum", bufs=2, space="PSUM") as ap,
        ):
            pending = []   # queued closures taking (scratch_psum or None)

            for h in range(2):
                av_ts = [ap.tile([112, MC], F32, tag="av", name=f"av{h}_{t}")
                         for t in range(2)]
                started = [[False] * 4 for _ in range(2)]

                def make_round(av_ts_, started_, t, vls, els, is_last):
                    def go(scratch):
                        gl = len(vls)
                        for j in range(gl):
                            for s in range(4):
                                first = not started_[t][s]
                                started_[t][s] = True
                                nc.tensor.matmul(
                                    av_ts_[t][ds(32 * s, D), :],
                                    vls[j][:],
                                    els[j][:, ts(4 * t + s, MC)],
                                    start=first,
                                    stop=(is_last and j == gl - 1),
                                    tile_position=(0, 32 * s))
                    return go

                def make_evac(h_, av_ts_):
                    def go(scratch):
                        for t in range(2):
                            nc.vector.tensor_copy(
                                out_sb[h_][:, ts(t, MC)], av_ts_[t][:])
                    return go

                def make_proj0(mc):
                    def go(scratch):
                        s, t = mc % 4, mc // 4
                        yp = scratch[ds(0, C), ds(2 * SCH[0] // 3, MC)]
                        nc.tensor.matmul(
                            yp, wp_t[0][ds(32 * s, D), :],
                            out_sb[0][ds(32 * s, D), ts(t, MC)],
                            start=True, stop=True,
                            tile_position=(32 * s, 0))
                        nc.vector.tensor_copy(y_sb[:, ts(mc, MC)], yp)
                    return go

                nt0 = 0
                for g, gsz in enumerate(GROUPS):
                    e_tiles, vts_tiles = [], []
                    for j in range(gsz):
                        nt = nt0 + j
                        e_t = ep.tile([NT, N], BF16, tag="e",
                                      name=f"e{h}_{nt}")
                        off = 0
                        s_ps3 = None
                        for ci, csz in enumerate(SCH):
                            s_ps = sp.tile([NT, SCH[0]], F32, tag="sa",
                                           name="s_ps")
                            for i in range(csz // MC):
                                if i % 2 == 1:
                                    b0, tp = 64, (64, 0)
                                else:
                                    b0, tp = 0, (0, 0)
                                nc.tensor.matmul(
                                    s_ps[:, ts(i, MC)],
                                    qsp[ds(b0, 3 * D),
                                        ds(h * N + nt * NT, NT)],
                                    ksp[ds(b0, 3 * D),
                                        ds(h * N + off + i * MC, MC)],
                                    start=True, stop=True, tile_position=tp)
                            if ci == 2:
                                s_ps3 = s_ps
                                # v matmuls piggyback on the idle 3rd bank
                                if h == 0 and nt < 8:
                                    for k in range(4):
                                        nv = 4 * nt + k
                                        psv = s_ps[:, ds(SCH[2] + 32 * k,
                                                         2 * D)]
                                        nc.tensor.matmul(
                                            psv, x1_t[:, ts(nv, NT)],
                                            wv_t[:], start=True, stop=True)
                                        nc.vector.tensor_copy(
                                            vT_sb[:, ts(nv, 2 * D)], psv)
                            nc.scalar.activation(
                                e_t[:, ds(off, csz)], s_ps[:, :csz], AF.Exp)
                            off += csz
                        if pending:
                            pending.pop(0)(s_ps3)
                        # rowsum: bf16 add-tree (TT runs 2x for bf16, the
                        # plain 4096-col reduce would be 1x / 4.4us).
                        # Level 1 alternates gpsimd/DVE for load balance.
                        tm1 = rp.tile([NT, N // 2], BF16, tag="tm1",
                                      name="tm1", bufs=2)
                        eng = nc.gpsimd if nt % 2 == 0 else nc.vector
                        eng.tensor_add(tm1[:], e_t[:, 0:N // 2],
                                       e_t[:, N // 2:N])
                        tm2 = rp.tile([NT, N // 4], BF16, tag="tm2",
                                      name="tm2", bufs=2)
                        nc.vector.tensor_add(tm2[:], tm1[:, 0:N // 4],
                                             tm1[:, N // 4:N // 2])
                        rs = rp.tile([NT, 1], F32, tag="rs", name="rs")
                        nc.vector.reduce_sum(
                            rs[:], tm2[:], axis=mybir.AxisListType.X)
                        rinv = rp.tile([NT, 1], F32, tag="ri", name="rinv")
                        nc.vector.reciprocal(rinv[:], rs[:])
                        vts = vp.tile([NT, D], BF16, tag="vts",
                                      name=f"vts{h}_{nt}")
                        nc.gpsimd.tensor_scalar_mul(
                            vts[:], vT_sb[:, ds(nt * 2 * D + h * D, D)],
                            rinv[:])
                        e_tiles.append(e_t)
                        vts_tiles.append(vts)
                    is_last = g == len(GROUPS) - 1
                    for t in range(2):
                        pending.append(make_round(
                            av_ts, started, t, vts_tiles, e_tiles, is_last))
                    if is_last:
                        pending.append(make_evac(h, av_ts))
                        if h == 0:
                            for mc in range(MCN):
                                pending.append(make_proj0(mc))
                    nt0 += gsz

            # ---- tail: flush queue, then head-1 projection + output ----
            while pending:
                scr = sp.tile([NT, SCH[0]], F32, tag="sa", name="scr")
                pending.pop(0)(scr)
            for mc in range(MCN):
                s, t = mc % 4, mc // 4
                scr = sp.tile([NT, SCH[0]], F32, tag="sa", name="scr2")
                yp = scr[ds(0, C), ds(2 * SCH[0] // 3, MC)]
                nc.tensor.matmul(
                    yp, wp_t[1][ds(32 * s, D), :],
                    out_sb[1][ds(32 * s, D), ts(t, MC)],
                    start=True, stop=True, tile_position=(32 * s, 0))
                nc.vector.tensor_add(
                    y2_sb[:, ts(mc, MC)], y_sb[:, ts(mc, MC)], yp)
                nc.sync.dma_start(y[:, ts(mc, MC)], y2_sb[:, ts(mc, MC)])


_PROGRAM = None


def _get_program():
    global _PROGRAM
    if _PROGRAM is None:
        nc = bacc.Bacc("TRN2", target_bir_lowering=False, debug=False,
                       num_devices=8)
        args = {}
        for nm, shape in (("x1", [C + 1, N]), ("wqh", [C + 1, 2 * D]),
                          ("wql", [C + 1, 2 * D]), ("wkh", [C + 1, 2 * D]),
                          ("wkl", [C + 1, 2 * D]), ("wv", [C + 1, 2 * D]),
                          ("wp0", [112, C]), ("wp1", [112, C])):
            args[nm] = nc.dram_tensor(nm, shape, F32,
                                      kind="ExternalInput").ap()
        yt = nc.dram_tensor("y", [C, N], F32, kind="ExternalOutput").ap()
        with tile.TileContext(nc) as tc:
            _body(tc, yt, **args)
        nc.compile()
        _PROGRAM = nc
    return _PROGRAM


def _bf16_round(a):
    u = np.ascontiguousarray(a, dtype=np.float32).view(np.uint32)
    lsb = (u >> 16) & 1
    u2 = u + 0x7FFF + lsb
    return (u2 & 0xFFFF0000).astype(np.uint32).view(np.float32)


def _make_in_maps(x, qkv_w, qkv_b, proj_w, proj_b=None):
    x = np.asarray(x, dtype=np.float32)
    qkv_w = np.asarray(qkv_w, dtype=np.float32)
    qkv_b = np.asarray(qkv_b, dtype=np.float32)
    proj_w = np.asarray(proj_w, dtype=np.float32)

    in_maps = []
    for core in range(8):
        b = core // 2
        h0 = 2 * (core % 2)
        heads = (h0, h0 + 1)
        x1 = np.concatenate(
            [x[b].reshape(C, N), np.ones((1, N), np.float32)], axis=0)

        def aug_qk(block):
            w = np.empty((C + 1, 2 * D), np.float32)
            for j, h in enumerate(heads):
                rows = slice(block * C + h * D, block * C + (h + 1) * D)
                w[:C, j * D:(j + 1) * D] = qkv_w[rows, :].T
                w[C, j * D:(j + 1) * D] = qkv_b[rows]
            return w

        def wp_rep(h):
            w = np.zeros((112, C), np.float32)
            blk = proj_w[:, h * D:(h + 1) * D].T  # [D, C]
            for s in range(4):
                w[32 * s:32 * s + D, :] = blk
            return w

        wq_a, wk_a = aug_qk(0), aug_qk(1)
        wq_h, wk_h = _bf16_round(wq_a), _bf16_round(wk_a)

        in_maps.append({
            "x1": np.ascontiguousarray(x1),
            "wqh": wq_h, "wql": wq_a - wq_h,
            "wkh": wk_h, "wkl": wk_a - wk_h,
            "wv": aug_qk(2),
            "wp0": wp_rep(heads[0]),
            "wp1": wp_rep(heads[1]),
        })
    return in_maps


def run_cores(inputs, **kw):
    """Compile+run on the 8 cores; returns BassKernelResults."""
    nc = _get_program()
    in_maps = _make_in_maps(**inputs)
    return run_bass_kernel_spmd(nc, in_maps, list(range(8)), **kw)


def kernel(x, qkv_w, qkv_b, proj_w, proj_b):
    res = run_cores(dict(x=x, qkv_w=qkv_w, qkv_b=qkv_b,
                         proj_w=proj_w, proj_b=proj_b))
    proj_b = np.asarray(proj_b, dtype=np.float32)
    parts = [r["y"] for r in res.results]
    out = np.empty((B, C, N), np.float32)
    for b in range(B):
        out[b] = parts[2 * b] + parts[2 * b + 1] + proj_b[:, None]
    return out.reshape(B, C, 64, 64)


if __name__ == "__main__":
    _get_program()
    print("program built OK")


# revision 13
# speedup vs baseline: 1.2336x; 1.2336x over previous
"""Trainium2 Bass kernel for nn_Attention_82540681494971.

Spatial self-attention block (LDM AttnBlock style, unscaled):
  qkv = conv1x1(x);  s = q^T k  [n x n] per (b,head);  attn = softmax(s, axis=-1)
  out[d,m] = sum_n v[d,n] attn[n,m];  y = conv1x1(out)

Shapes: B=4, C=64, H=W=64 -> n=4096 tokens, HEAD=4, d=16.

Sharding: 8 cores, core c handles batch b=c//2 and heads (0,1) if c%2==0
else (2,3). Each core computes a partial projection output over its two
heads' channels; host sums the two partials per batch and adds proj bias.

Key algebra: attn[n,m] = E[n,m]/rowsum[n] with E=exp(s). Since the AV
contraction runs over n (the softmax row index), fold 1/rowsum into v:
  out[d,m] = sum_n (v[d,n]*rinv[n]) E[n,m]
so the big E matrix never needs normalizing. Scores are exact-fp32-grade
via a 3-term bf16 split (q=q_hi+q_lo, k=k_hi+k_lo, drop lo*lo):
  s = [q_hi;q_lo;q_hi]^T [k_hi;k_hi;k_lo]   (K=48 stacked, 1 cyc/row)

v2 design (ACT-roofline targeted):
  - ACT does exp ONLY: rowsums split across DVE and GpSimd reduces over
    the bf16 E tile (no ACT accum_out reads).
  - AV runs 4 concurrent PSUM-accumulation chains in PE col-groups
    (tile_position=(0,32s)); strip s owns m-chunks {s, 4+s}. The two
    [112,512] PSUM accumulators persist across all groups of a head.
  - qkv matmuls in bf16 via a host-side 3-term weight split (W_hi x_hi +
    W_hi x_lo + W_lo x_hi, bias exact via hi/lo bias rows) - denser PE
    stream than fp32, warms the HAM clock gate early.
  - v matmuls and head-0 projection woven into phase 1, using the idle
    3rd PSUM bank of chunk-3 score slots as scratch.
  - Projection col-tiled over 4 row-groups in bf16, split per head:
    h0 during h1's steady state, h1 at the tail (y = y0 + yp1 on DVE).
  - PSUM: 6 banks scores (2 x [128,1536] double-buffer) + 2 banks AV.
"""

import numpy as np
from contextlib import ExitStack

import concourse.bass as bass
import concourse.mybir as mybir
import concourse.tile as tile
from concourse import bacc
from concourse.bass import ts, ds
from concourse.bass_utils import run_bass_kernel_spmd

F32 = mybir.dt.float32
BF16 = mybir.dt.bfloat16
AF = mybir.ActivationFunctionType

B, C, HEAD, D = 4, 64, 4, 16
N = 4096          # tokens = H*W
NT = 128          # n-tile (partition) size
NTILES = N // NT  # 32
MC = 512          # matmul free-dim chunk
MCN = N // MC     # 8 m-chunks
SCH = (1536, 1536, 1024)  # scores/exp PSUM chunking (3 banks/slot, 2 bufs)
GROUPS = [4] * 7 + [2, 2]  # AV group sizes (short tail for a fast finish)


def _body(tc, y, x1, wqh, wql, wkh, wkl, wv, wp0, wp1):
    nc = tc.nc
    ctx = ExitStack()
    with ctx:
        pp = ctx.enter_context(tc.tile_pool(name="persist", bufs=1))
        cp = ctx.enter_context(tc.tile_pool(name="consts", bufs=1))

        # ---- warm the exp table while DMAs run ----
        zz = pp.tile([NT, 1], F32)
        zz2 = pp.tile([NT, 1], F32)
        nc.gpsimd.memset(zz[:], 0.0)
        nc.scalar.activation(zz2[:], zz[:], AF.Exp)

        # ---- constants ----
        wf = {}
        for name, src in (("qh", wqh), ("ql", wql), ("kh", wkh), ("kl", wkl)):
            t = cp.tile([C + 1, 2 * D], F32, tag="wf", name=f"wf_{name}",
                        bufs=4)
            nc.sync.dma_start(t[:], src[:])
            wf[name] = t
        wv_t = cp.tile([C + 1, 2 * D], F32)
        nc.sync.dma_start(wv_t[:], wv[:])
        wp0_f = cp.tile([112, C], F32)
        wp1_f = cp.tile([112, C], F32)
        nc.sync.dma_start(wp0_f[:], wp0[:])
        nc.sync.dma_start(wp1_f[:], wp1[:])
        wb = {}
        for name in ("qh", "ql", "kh", "kl"):
            t = cp.tile([C + 1, 2 * D], BF16, tag="wb", name=f"wb_{name}",
                        bufs=4)
            nc.vector.tensor_copy(t[:], wf[name][:])
            wb[name] = t
        wp_t = [cp.tile([112, C], BF16, tag="wpb", name=f"wpb{h}", bufs=2)
                for h in range(2)]
        nc.vector.tensor_copy(wp_t[0][:], wp0_f[:])
        nc.vector.tensor_copy(wp_t[1][:], wp1_f[:])

        # ---- persistent SBUF ----
        x1_t = pp.tile([C + 1, N], F32)   # kept for v matmuls in phase 1
        qsp = pp.tile([112, 2 * N], BF16)
        ksp = pp.tile([112, 2 * N], BF16)
        vT_sb = pp.tile([NT, NTILES * 2 * D], F32)  # per n-tile [128,32] h0|h1
        out_sb = [pp.tile([112, 2 * MC], BF16, tag=f"osb{h}", name=f"osb{h}")
                  for h in range(2)]
        y_sb = pp.tile([C, N], F32)    # head-0 projection partial
        y2_sb = pp.tile([C, N], F32)   # final y = y_sb + yp1


        # ---- phase 0: q,k in bf16 (3-term) + hi/lo split + K=48 stacks ----
        with (
            tc.tile_pool(name="x1p", bufs=1) as xp,
            tc.tile_pool(name="qkf", bufs=1) as qf,
            tc.tile_pool(name="spl", bufs=1) as spl,
            tc.tile_pool(name="p0psum", bufs=5, space="PSUM") as p0,
        ):
            for i in range(8):  # parallel DMA queues
                nc.sync.dma_start(x1_t[:, ts(i, N // 8)], x1[:, ts(i, N // 8)])
            x_hi = xp.tile([C + 1, N], BF16)
            x_lo = xp.tile([C + 1, N], BF16)
            for i in range(4):
                sl = ts(i, N // 4)
                nc.vector.tensor_copy(x_hi[:, sl], x1_t[:, sl])
                eng = nc.vector if i < 2 else nc.gpsimd
                eng.tensor_sub(x_lo[:, sl], x1_t[:, sl], x_hi[:, sl])

            for which, dst, dup in (("q", qsp, 2), ("k", ksp, 1)):
                w_hi, w_lo = wb[which[0] + "h"], wb[which[0] + "l"]
                f32_sb = qf.tile([D, 2 * N], F32, tag="qkf32")
                for h in range(2):
                    for mc in range(MCN):
                        ps = p0.tile([D, MC], F32, tag="p0")
                        nc.tensor.matmul(
                            ps[:], w_hi[:, ts(h, D)], x_hi[:, ts(mc, MC)],
                            start=True, stop=False)
                        nc.tensor.matmul(
                            ps[:], w_hi[:, ts(h, D)], x_lo[:, ts(mc, MC)],
                            start=False, stop=False)
                        nc.tensor.matmul(
                            ps[:], w_lo[:, ts(h, D)], x_hi[:, ts(mc, MC)],
                            start=False, stop=True)
                        dst_ap = f32_sb[:, ds(h * N + mc * MC, MC)]
                        if mc % 2 == 0:
                            nc.vector.tensor_copy(dst_ap, ps[:])
                        else:
                            nc.scalar.copy(dst_ap, ps[:])
                hi_t = spl.tile([D, 2 * N], BF16, tag="hi")
                lo_t = spl.tile([D, 2 * N], BF16, tag="lo")
                # hi-cast on ACT, lo-sub on DVE (gpsimd is slow on big ops)
                nc.scalar.copy(hi_t[:, 0:N], f32_sb[:, 0:N])
                nc.scalar.copy(hi_t[:, N:2 * N], f32_sb[:, N:2 * N])
                nc.vector.tensor_sub(lo_t[:, 0:N], f32_sb[:, 0:N],
                                     hi_t[:, 0:N])
                nc.vector.tensor_sub(lo_t[:, N:2 * N], f32_sb[:, N:2 * N],
                                     hi_t[:, N:2 * N])
                # assemble K=48 stack via SBUF->SBUF DMA
                lo_block = 1 if which == "q" else 2
                for i in range(4):
                    sl = ts(i, N // 2)
                    for b0 in (0, 64):
                        nc.sync.dma_start(dst[ds(b0, D), sl], hi_t[:, sl])
                        nc.sync.dma_start(
                            dst[ds(b0 + lo_block * D, D), sl], lo_t[:, sl])
                        nc.sync.dma_start(
                            dst[ds(b0 + dup * D, D), sl], hi_t[:, sl])

        # ---- phase 1: attention, software-pipelined ----
        # Per n-tile: 3 score-chunk matmul bursts feed 3 exp ACTIVATEs (ACT
        # exp-only). Rowsum split DVE+GpSimd from the bf16 E tile. Queued
        # work (AV rounds of the previous group, accum evacs, head-0
        # projection) pops once per n-tile with the chunk-3 psum slot's
        # idle 3rd bank as scratch. v matmuls for n-tile 4i..4i+3 run in
        # n-tile i's chunk-3 slot (first 8 n-tiles of head 0).
        with (
            tc.tile_pool(name="ep", bufs=11) as ep,
            tc.tile_pool(name="rp", bufs=6) as rp,
            tc.tile_pool(name="vp", bufs=10) as vp,
            tc.tile_pool(name="sapsum", bufs=2, space="PSUM") as sp,
            tc.tile_pool(name="avpsum", bufs=2, space="PSUM") as ap,
        ):
            pending = []   # queued closures taking (scratch_psum or None)

            for h in range(2):
                av_ts = [ap.tile([112, MC], F32, tag="av", name=f"av{h}_{t}")
                         for t in range(2)]
                started = [[False] * 4 for _ in range(2)]

                def make_round(av_ts_, started_, t, vls, els, is_last):
                    def go(scratch):
                        gl = len(vls)
                        for j in range(gl):
                            for s in range(4):
                                first = not started_[t][s]
                                started_[t][s] = True
                                nc.tensor.matmul(
                                    av_ts_[t][ds(32 * s, D), :],
                                    vls[j][:],
                                    els[j][:, ts(4 * t + s, MC)],
                                    start=first,
                                    stop=(is_last and j == gl - 1),
                                    tile_position=(0, 32 * s))
                    return go

                def make_evac(h_, av_ts_):
                    def go(scratch):
                        for t in range(2):
                            nc.vector.tensor_copy(
                                out_sb[h_][:, ts(t, MC)], av_ts_[t][:])
                    return go

                def make_proj0(mc):
                    def go(scratch):
                        s, t = mc % 4, mc // 4
                        yp = scratch[ds(0, C), ds(2 * SCH[0] // 3, MC)]
                        nc.tensor.matmul(
                            yp, wp_t[0][ds(32 * s, D), :],
                            out_sb[0][ds(32 * s, D), ts(t, MC)],
                            start=True, stop=True,
                            tile_position=(32 * s, 0))
                        nc.vector.tensor_copy(y_sb[:, ts(mc, MC)], yp)
                    return go

                nt0 = 0
                for g, gsz in enumerate(GROUPS):
                    e_tiles, vts_tiles = [], []
                    for j in range(gsz):
                        nt = nt0 + j
                        e_t = ep.tile([NT, N], BF16, tag="e",
                                      name=f"e{h}_{nt}")
                        off = 0
                        s_ps3 = None
                        for ci, csz in enumerate(SCH):
                            s_ps = sp.tile([NT, SCH[0]], F32, tag="sa",
                                           name="s_ps")
                            for i in range(csz // MC):
                                if i % 2 == 1:
                                    b0, tp = 64, (64, 0)
                                else:
                                    b0, tp = 0, (0, 0)
                                nc.tensor.matmul(
                                    s_ps[:, ts(i, MC)],
                                    qsp[ds(b0, 3 * D),
                                        ds(h * N + nt * NT, NT)],
                                    ksp[ds(b0, 3 * D),
                                        ds(h * N + off + i * MC, MC)],
                                    start=True, stop=True, tile_position=tp)
                            if ci == 2:
                                s_ps3 = s_ps
                                # v matmuls piggyback on the idle 3rd bank
                                if h == 0 and nt < 8:
                                    for k in range(4):
                                        nv = 4 * nt + k
                                        psv = s_ps[:, ds(SCH[2] + 32 * k,
                                                         2 * D)]
                                        nc.tensor.matmul(
                                            psv, x1_t[:, ts(nv, NT)],
                                            wv_t[:], start=True, stop=True)
                                        nc.vector.tensor_copy(
                                            vT_sb[:, ts(nv, 2 * D)], psv)
                            nc.scalar.activation(
                                e_t[:, ds(off, csz)], s_ps[:, :csz], AF.Exp)
                            off += csz
                        if pending:
                            pending.pop(0)(s_ps3)
                        # rowsum: 3-level bf16 add-tree on DVE (TT runs 2x
                        # for bf16; a plain 4096-col reduce would be 4.4us)
                        tm1 = rp.tile([NT, N // 2], BF16, tag="tm1",
                                      name="tm1", bufs=2)
                        nc.vector.tensor_add(tm1[:], e_t[:, 0:N // 2],
                                             e_t[:, N // 2:N])
                        tm2 = rp.tile([NT, N // 4], BF16, tag="tm2",
                                      name="tm2", bufs=2)
                        nc.vector.tensor_add(tm2[:], tm1[:, 0:N // 4],
                                             tm1[:, N // 4:N // 2])
                        tm3 = rp.tile([NT, N // 8], BF16, tag="tm3",
                                      name="tm3", bufs=2)
                        nc.vector.tensor_add(tm3[:], tm2[:, 0:N // 8],
                                             tm2[:, N // 8:N // 4])
                        rs = rp.tile([NT, 1], F32, tag="rs", name="rs")
                        nc.vector.reduce_sum(
                            rs[:], tm3[:], axis=mybir.AxisListType.X)
                        rinv = rp.tile([NT, 1], F32, tag="ri", name="rinv")
                        nc.vector.reciprocal(rinv[:], rs[:])
                        vts = vp.tile([NT, D], BF16, tag="vts",
                                      name=f"vts{h}_{nt}")
                        nc.gpsimd.tensor_scalar_mul(
                            vts[:], vT_sb[:, ds(nt * 2 * D + h * D, D)],
                            rinv[:])
                        e_tiles.append(e_t)
                        vts_tiles.append(vts)
                    is_last = g == len(GROUPS) - 1
                    for t in range(2):
                        pending.append(make_round(
                            av_ts, started, t, vts_tiles, e_tiles, is_last))
                    if is_last:
                        pending.append(make_evac(h, av_ts))
                        if h == 0:
                            for mc in range(MCN):
                                pending.append(make_proj0(mc))
                    nt0 += gsz

            # ---- tail: flush queue, then head-1 projection + output ----
            while pending:
                pending.pop(0)(None)
            for mc in range(MCN):
                s, t = mc % 4, mc // 4
                scr = sp.tile([NT, SCH[0]], F32, tag="sa", name="scr2")
                yp = scr[ds(0, C), ds(2 * SCH[0] // 3, MC)]
                nc.tensor.matmul(
                    yp, wp_t[1][ds(32 * s, D), :],
                    out_sb[1][ds(32 * s, D), ts(t, MC)],
                    start=True, stop=True, tile_position=(32 * s, 0))
                nc.vector.tensor_add(
                    y2_sb[:, ts(mc, MC)], y_sb[:, ts(mc, MC)], yp)
                nc.sync.dma_start(y[:, ts(mc, MC)], y2_sb[:, ts(mc, MC)])


_PROGRAM = None


def _get_program():
    global _PROGRAM
    if _PROGRAM is None:
        nc = bacc.Bacc("TRN2", target_bir_lowering=False, debug=False,
                       num_devices=8)
        args = {}
        for nm, shape in (("x1", [C + 1, N]), ("wqh", [C + 1, 2 * D]),
                          ("wql", [C + 1, 2 * D]), ("wkh", [C + 1, 2 * D]),
                          ("wkl", [C + 1, 2 * D]), ("wv", [C + 1, 2 * D]),
                          ("wp0", [112, C]), ("wp1", [112, C])):
            args[nm] = nc.dram_tensor(nm, shape, F32,
                                      kind="ExternalInput").ap()
        yt = nc.dram_tensor("y", [C, N], F32, kind="ExternalOutput").ap()
        with tile.TileContext(nc) as tc:
            _body(tc, yt, **args)
        nc.compile()
        _PROGRAM = nc
    return _PROGRAM


def _bf16_round(a):
    u = np.ascontiguousarray(a, dtype=np.float32).view(np.uint32)
    lsb = (u >> 16) & 1
    u2 = u + 0x7FFF + lsb
    return (u2 & 0xFFFF0000).astype(np.uint32).view(np.float32)


def _make_in_maps(x, qkv_w, qkv_b, proj_w, proj_b=None):
    x = np.asarray(x, dtype=np.float32)
    qkv_w = np.asarray(qkv_w, dtype=np.float32)
    qkv_b = np.asarray(qkv_b, dtype=np.float32)
    proj_w = np.asarray(proj_w, dtype=np.float32)

    in_maps = []
    for core in range(8):
        b = core // 2
        h0 = 2 * (core % 2)
        heads = (h0, h0 + 1)
        x1 = np.concatenate(
            [x[b].reshape(C, N), np.ones((1, N), np.float32)], axis=0)

        def aug_qk(block):
            w = np.empty((C + 1, 2 * D), np.float32)
            for j, h in enumerate(heads):
                rows = slice(block * C + h * D, block * C + (h + 1) * D)
                w[:C, j * D:(j + 1) * D] = qkv_w[rows, :].T
                w[C, j * D:(j + 1) * D] = qkv_b[rows]
            return w

        def wp_rep(h):
            w = np.zeros((112, C), np.float32)
            blk = proj_w[:, h * D:(h + 1) * D].T  # [D, C]
            for s in range(4):
                w[32 * s:32 * s + D, :] = blk
            return w

        wq_a, wk_a = aug_qk(0), aug_qk(1)
        wq_h, wk_h = _bf16_round(wq_a), _bf16_round(wk_a)

        in_maps.append({
            "x1": np.ascontiguousarray(x1),
            "wqh": wq_h, "wql": wq_a - wq_h,
            "wkh": wk_h, "wkl": wk_a - wk_h,
            "wv": aug_qk(2),
            "wp0": wp_rep(heads[0]),
            "wp1": wp_rep(heads[1]),
        })
    return in_maps


def run_cores(inputs, **kw):
    """Compile+run on the 8 cores; returns BassKernelResults."""
    nc = _get_program()
    in_maps = _make_in_maps(**inputs)
    return run_bass_kernel_spmd(nc, in_maps, list(range(8)), **kw)


def kernel(x, qkv_w, qkv_b, proj_w, proj_b):
    res = run_cores(dict(x=x, qkv_w=qkv_w, qkv_b=qkv_b,
                         proj_w=proj_w, proj_b=proj_b))
    proj_b = np.asarray(proj_b, dtype=np.float32)
    parts = [r["y"] for r in res.results]
    out = np.empty((B, C, N), np.float32)
    for b in range(B):
        out[b] = parts[2 * b] + parts[2 * b + 1] + proj_b[:, None]
    return out.reshape(B, C, 64, 64)


if __name__ == "__main__":
    _get_program()
    print("program built OK")
